# revision 22
# baseline (speedup 1.0000x reference)
"""FFM layer (embedding lookup + field-factorization) on 8 trn2 NeuronCores.

Strategy: data-parallel over batch (4096 rows -> 512/core), embedding tables
replicated to every core.  The reference's inner reduction
  latent_sum[b,f,k] = sum_j v[idx[b,f], j, k]
sums over ALL 26 fields j regardless of the batch indices, so
  vred[i,k] = sum_j v[i,j,k]
is a pure function of the parameters and is folded into the table host-side
(same spirit as packing w into the augmented table).  Likewise the
second-order self term and first-order weight fold into one row scalar
  c[i] = w[i] + w0/26 - 0.5*|vred[i]|^2,
leaving the device with
  out[b] = sum_f c[idx[b,f]] + 0.5 * |sum_f vred[idx[b,f]]|^2.

Each table row is [vred (8 f32) | c | pad] = 64 f32 = 256 B, the SWDGE
minimum elem size -- 4x fewer gathered bytes than the 1 KiB rows of the
naive packing, and no on-device j-reduction at all.  Lookups use the SWDGE
dma_gather custom instruction, one per field (field-local int16 indices,
512 per gather).  Index ordinal i = batch row lands at dest
[i % 128, i // 128, :], exactly the (partition, batch-tile) layout the
VectorE tail wants.
"""

import sys

import numpy as np

FIELD = 26
K = 8
RPAD = 64                # padded row length in f32 (256 B, SWDGE minimum)
VOCAB = 20000
TOTAL = FIELD * VOCAB    # 520000
B = 4096
NCORES = 8
BC = B // NCORES         # 512 batch rows per core
P = 128
NTILES = BC // P         # 4
NSLOT = BC // 16         # 32 int16 index slots per idx partition

_TRN_REPO = "/opt/trn_rl_repo"

_cache = {}


def _build_nc(n_iters=1):
    if _TRN_REPO not in sys.path:
        sys.path.insert(0, _TRN_REPO)
    from concourse import bacc, mybir, tile

    f32 = mybir.dt.float32
    i16 = mybir.dt.int16
    Alu = mybir.AluOpType
    Ax = mybir.AxisListType

    nc = bacc.Bacc("TRN2", target_bir_lowering=False, debug=False)
    # idx16[p, f, s] = int16 field-local index of batch row s*16+(p%16),
    # field f -- 16-partition wrap replicated to 128 host-side
    idx_d = nc.dram_tensor("idx16", [P, FIELD, NSLOT], i16,
                           kind="ExternalInput")
    tab_d = nc.dram_tensor("tab", [TOTAL, RPAD], f32, kind="ExternalInput")
    out_d = nc.dram_tensor("out", [BC, 1], f32, kind="ExternalOutput")

    with tile.TileContext(nc) as tc:
        with tc.tile_pool(name="pool", bufs=1) as pool:
            for _ in range(n_iters):
                idx_sb = pool.tile([P, FIELD, NSLOT], i16, tag="idx")
                nc.sync.dma_start(out=idx_sb[:], in_=idx_d[:, :, :])

                # vg[p, f, t, :] = tab[f*VOCAB + idx[t*128+p, f], :]
                # NOTE: each dma_gather blocks the Pool engine for
                # ~994ns + 7ns/descriptor of SWDGE desc-gen (measured);
                # 26 x 4.6us is the kernel's critical path.  prepare_only
                # preps cost the same, so prep/trigger pipelining and
                # queue spreading don't help.
                vg = pool.tile([P, FIELD, NTILES, RPAD], f32, tag="vg")
                for f in range(FIELD):
                    nc.gpsimd.dma_gather(
                        out_ap=vg[:, f],
                        in_ap=tab_d[f * VOCAB:(f + 1) * VOCAB, :],
                        idxs_ap=idx_sb[:, f, :],
                        num_idxs=BC,
                        num_idxs_reg=BC,
                        elem_size=RPAD,
                    )

                # s[p, t, k] = sum_f vred[idx, k]
                s_all = pool.tile([P, NTILES, K], f32, tag="s")
                nc.vector.tensor_reduce(
                    out=s_all[:],
                    in_=vg[:, :, :, 0:K].rearrange("p f t k -> p t k f"),
                    axis=Ax.X,
                    op=Alu.add,
                )
                # csum[p, t] = sum_f c[idx]
                csum = pool.tile([P, NTILES], f32, tag="c")
                nc.vector.tensor_reduce(
                    out=csum[:],
                    in_=vg[:, :, :, K].rearrange("p f t -> p t f"),
                    axis=Ax.X,
                    op=Alu.add,
                )
                ssq = pool.tile([P, NTILES, K], f32, tag="ssq")
                nc.vector.tensor_tensor(
                    out=ssq[:], in0=s_all[:], in1=s_all[:], op=Alu.mult
                )
                s2 = pool.tile([P, NTILES], f32, tag="s2")
                nc.vector.tensor_reduce(
                    out=s2[:], in_=ssq[:], axis=Ax.X, op=Alu.add
                )
                s2h = pool.tile([P, NTILES], f32, tag="s2h")
                nc.vector.tensor_scalar_mul(s2h[:], s2[:], 0.5)
                out_all = pool.tile([P, NTILES], f32, tag="oa")
                nc.vector.tensor_tensor(
                    out=out_all[:], in0=s2h[:], in1=csum[:], op=Alu.add
                )
                # single store: out[t*128+p] = out_all[p, t]
                nc.sync.dma_start(
                    out=out_d[:, :].rearrange("(t p) one -> p (t one)", p=P),
                    in_=out_all[:],
                )
    nc.compile()
    return nc


def get_nc():
    if "nc" not in _cache:
        _cache["nc"] = _build_nc()
    return _cache["nc"]


def make_in_maps(inputs, offsets, w0, w, v):
    del offsets  # folded into the per-field subtable slicing
    inp = np.asarray(inputs)
    idx16 = np.ascontiguousarray(
        inp.astype(np.int16).reshape(NCORES, BC, FIELD)
    )
    # reduced table row: [vred (8 f32) | c | pad to 64 f32 = 256 B]
    vred = np.asarray(v, dtype=np.float32).reshape(TOTAL, FIELD, K).sum(axis=1)
    c = (np.asarray(w, dtype=np.float32).reshape(TOTAL)
         + np.float32(np.asarray(w0, np.float32).reshape(()) / FIELD)
         - 0.5 * (vred * vred).sum(axis=1))
    tab = np.zeros((TOTAL, RPAD), dtype=np.float32)
    tab[:, :K] = vred
    tab[:, K] = c
    maps = []
    for i in range(NCORES):
        shard = idx16[i]                       # [BC, FIELD]
        wrapped = shard.reshape(NSLOT, 16, FIELD).transpose(1, 2, 0)
        # [16, FIELD, NSLOT] -> replicate to 128 partitions
        rep = np.ascontiguousarray(np.tile(wrapped, (NCORES, 1, 1)))
        maps.append({"idx16": rep, "tab": tab})
    return maps


def assemble_out(res):
    return np.concatenate(
        [np.asarray(res.results[i]["out"]) for i in range(NCORES)], axis=0
    ).astype(np.float32)


def kernel(inputs, offsets, w0, w, v):
    if _TRN_REPO not in sys.path:
        sys.path.insert(0, _TRN_REPO)
    from concourse.bass_utils import run_bass_kernel_spmd

    nc = get_nc()
    in_maps = make_in_maps(inputs, offsets, w0, w, v)
    res = run_bass_kernel_spmd(nc, in_maps, list(range(NCORES)))
    return assemble_out(res)
